# revision 28
# baseline (speedup 1.0000x reference)
"""RBF/KNN interpolation kernel for Trainium2 (8 NeuronCores, data parallel).

Computes, per batch b:
    v        = input_data[b, -1, :, 0]                      (N_in,)
    w[o, i]  = exp(-||tc[o] - ic[i]||^2 / (2 * 0.1^2))      (N_out, N_in)
    interp   = (w @ v) / (w.sum(-1) + 1e-8)                 (N_out,)
    out[b]   = broadcast(interp) -> (n_samples, N_out, 4)

Sharding: batch B=8 across 8 cores (one batch per core). The weight matrix
is built on-chip tile by tile (never materialized in HBM):
  - logits psum[i, o] via a K=8 fp16 matmul. fp32 coords are split into
    fp16 (hi, lo) pairs so the single-pass fp16 PE path keeps ~1e-4
    precision on the exponent (fp32 matmul runs 2 passes at half rate):
      cross = xh*txh + xh*txl + xl*txh + (same for y) + 1*t2h + 1*t2l
    where t2h + t2l ~= -0.5 * |tc|^2.
  - w = Exp(100 * logits + bias[i]) on the scalar engine, written as fp16;
    bias = -50*|ic|^2 + 10*ln(2) (the 2^10 factor keeps small weights out
    of the fp16 denormal range; it cancels in num/den).
  - [num; ...; den] += [v, 0 x31, 1].T @ w  (fp16 matmul, fp32 psum accum;
    den lands on psum partition 32 - compute-engine PSUM APs need 32-aligned
    starts).
  - interp = num / (den + 1024e-8), computed in a [128, L] layout, then
    broadcast x4 (vector copies) and x n_samples (DMA) to the output.
"""

from contextlib import ExitStack
from functools import lru_cache

import numpy as np

import concourse.bass as bass
import concourse.bacc as bacc
import concourse.tile as tile
from concourse import mybir
from concourse.bass_utils import run_bass_kernel_spmd

F32 = mybir.dt.float32
F16 = mybir.dt.float16
AF = mybir.ActivationFunctionType
ALU = mybir.AluOpType

# Problem sizes (hardcoded per spec)
B = 8
T_IN = 4
N_IN = 4096
V_IN = 3
N_OUT = 8192
S = 10
T_OUT = 4
GAMMA = 50.0  # 1 / (2 * LENGTH_SCALE^2), LENGTH_SCALE = 0.1
EPS = 1e-8
WSCALE_LOG = 6.93147180559945  # ln(2^10)
WSCALE = 1024.0


def build_kernel(tc_ctx, dat, ic_h, tc_h, out_h, n_in, n_out, s, F=1024):
    tcx = tc_ctx
    nc = tcx.nc
    IC = n_in // 128   # i-chunks
    OC = n_out // F    # o-chunks
    NSUB = F // 512
    L = n_out // 128   # per-partition interp count in output layout
    CT = n_out // 128  # nat-layout columns (target side)

    with ExitStack() as ctx:
        const_pool = ctx.enter_context(tcx.tile_pool(name="const", bufs=1))

        # ---- persistent tiles ----
        # K is zero-padded 8 -> 128: a full-array matmul costs the same cycles
        # (stream rate is per column) but keeps the PE HAM activity monitor
        # seeing a busy array, so the clock un-throttles to 2.4 GHz.
        tc_aug = const_pool.tile([128, n_out], F16)  # rows t2h t2l txh txl txh tyh tyl tyh, rest 0
        ic_aug = const_pool.tile([128, n_in], F16)   # rows 1   1   xh  xh  xl  yh  yh  yl, rest 0
        bias_nat = const_pool.tile([128, IC], F32)
        vo_nat = const_pool.tile([128, 128 * IC], F16)  # [v, 0..., 1@32, 0...] per chunk
        nd_rows = const_pool.tile([33, n_out], F32)  # row 0 = num, row 32 = den
        ident = const_pool.tile([128, 128], F16)

        # ---- head: inputs, identity, splits (all in 128-partition nat layout) ----
        head = ctx.enter_context(tcx.tile_pool(name="head", bufs=1))
        tcx_nat = head.tile([128, CT], F32)
        tcy_nat = head.tile([128, CT], F32)
        icx_nat = head.tile([128, IC], F32)
        icy_nat = head.tile([128, IC], F32)
        v_nat = head.tile([128, IC], F32)

        # PE clock warm-up: the HAM monitor un-throttles (1.2 -> 2.4 GHz)
        # only after ~3.4us of sustained full-array work and re-throttles
        # after ~3.4us idle. Fill the PE's head idle time with dummy
        # full-array matmuls whose source is ready immediately, sized to end
        # roughly when the transpose inputs become ready.
        warm_cm = tcx.tile_pool(name="warm_ps", bufs=2, space="PSUM")
        warm_ps = warm_cm.__enter__()
        wsrc = head.tile([128, 512], F16)
        nc.gpsimd.memset(wsrc[:, :].bitcast(mybir.dt.uint32), 0)
        for _ in range(55):
            wps = warm_ps.tile([128, 512], F32, tag="warm")
            nc.tensor.matmul(wps[:, :], wsrc[:, 0:128], wsrc[:, :],
                             start=True, stop=True)

        # zero the padded-K operands first (they gate the row DMAs);
        # bitcast fp16 pairs to uint32 to halve the element count
        tc_aug_u = tc_aug[:, :].bitcast(mybir.dt.uint32)
        ic_aug_u = ic_aug[:, :].bitcast(mybir.dt.uint32)
        nc.vector.memset(tc_aug_u[:, :tc_aug_u.shape[1] // 2], 0)
        nc.gpsimd.memset(tc_aug_u[:, tc_aug_u.shape[1] // 2:], 0)
        nc.vector.memset(ic_aug_u, 0)

        # coordinate loads, one contiguous nat tile per component
        tc_r = tc_h[:].rearrange("(c p) d -> p c d", p=128)
        ic_r = ic_h[:].rearrange("(c p) d -> p c d", p=128)
        h = CT // 2
        nc.sync.dma_start(out=tcx_nat[:, 0:h].rearrange("p (c o) -> p c o", o=1),
                          in_=tc_r[:, 0:h, 0:1])
        nc.gpsimd.dma_start(out=tcx_nat[:, h:].rearrange("p (c o) -> p c o", o=1),
                            in_=tc_r[:, h:, 0:1])
        nc.scalar.dma_start(out=tcy_nat[:, 0:h].rearrange("p (c o) -> p c o", o=1),
                            in_=tc_r[:, 0:h, 1:2])
        nc.sync.dma_start(out=tcy_nat[:, h:].rearrange("p (c o) -> p c o", o=1),
                          in_=tc_r[:, h:, 1:2])
        nc.gpsimd.dma_start(out=icx_nat.rearrange("p (c o) -> p c o", o=1),
                            in_=ic_r[:, :, 0:1])
        nc.scalar.dma_start(out=icy_nat.rearrange("p (c o) -> p c o", o=1),
                            in_=ic_r[:, :, 1:2])
        nc.sync.dma_start(
            out=v_nat[:, :],
            in_=dat[:][T_IN - 1, :, 0].rearrange("(c p) -> p c", p=128),
        )

        # identity for PE transposes: ident[p, f] = (p == f)
        jj = head.tile([128, 128], F32)
        kk = head.tile([128, 1], F32)
        nc.gpsimd.iota(jj[:, :], [[1, 128]], base=0, channel_multiplier=0,
                       allow_small_or_imprecise_dtypes=True)
        nc.gpsimd.iota(kk[:, :], [[0, 1]], base=0, channel_multiplier=1,
                       allow_small_or_imprecise_dtypes=True)
        nc.gpsimd.tensor_scalar(ident[:, :], jj[:, :], kk[:, 0:1], None,
                                op0=ALU.is_equal)

        # --- target-side nat computes (chunk-major: x[p, c] = f(tc[c*128+p])) ---
        sqx_t = head.tile([128, CT], F32)
        t2s = head.tile([128, CT], F32)
        nc.gpsimd.tensor_mul(sqx_t[:, :], tcx_nat[:, :], tcx_nat[:, :])
        nc.vector.scalar_tensor_tensor(t2s[:, :], tcy_nat[:, :], 1.0,
                                       tcy_nat[:, :], op0=ALU.bypass,
                                       op1=ALU.mult)
        nc.vector.tensor_add(t2s[:, :], t2s[:, :], sqx_t[:, :])

        t2h_nat = head.tile([128, CT], F16)
        t2l_nat = head.tile([128, CT], F16)
        nc.vector.tensor_scalar_mul(t2h_nat[:, :], t2s[:, :], -0.5)
        nc.vector.scalar_tensor_tensor(t2l_nat[:, :], t2s[:, :], -0.5,
                                       t2h_nat[:, :], op0=ALU.mult,
                                       op1=ALU.subtract)
        txh_nat = head.tile([128, CT], F16)
        txl_nat = head.tile([128, CT], F16)
        tyh_nat = head.tile([128, CT], F16)
        tyl_nat = head.tile([128, CT], F16)
        nc.gpsimd.tensor_copy(txh_nat[:, :], tcx_nat[:, :])
        nc.vector.tensor_sub(txl_nat[:, :], tcx_nat[:, :], txh_nat[:, :])
        nc.gpsimd.tensor_copy(tyh_nat[:, :], tcy_nat[:, :])
        nc.vector.tensor_sub(tyl_nat[:, :], tcy_nat[:, :], tyh_nat[:, :])

        # --- input-side nat computes ---
        sqx_i = head.tile([128, IC], F32)
        i2s = head.tile([128, IC], F32)
        nc.gpsimd.tensor_mul(sqx_i[:, :], icx_nat[:, :], icx_nat[:, :])
        nc.vector.scalar_tensor_tensor(i2s[:, :], icy_nat[:, :], 1.0,
                                       icy_nat[:, :], op0=ALU.bypass,
                                       op1=ALU.mult)
        nc.vector.tensor_add(i2s[:, :], i2s[:, :], sqx_i[:, :])
        # bias = -50 * i2 + ln(2^10)
        nc.vector.tensor_scalar(bias_nat[:, :], i2s[:, :], -GAMMA, WSCALE_LOG,
                                op0=ALU.mult, op1=ALU.add)

        xh_nat = head.tile([128, IC], F16)
        xl_nat = head.tile([128, IC], F16)
        yh_nat = head.tile([128, IC], F16)
        yl_nat = head.tile([128, IC], F16)
        nc.gpsimd.tensor_copy(xh_nat[:, :], icx_nat[:, :])
        nc.vector.tensor_sub(xl_nat[:, :], icx_nat[:, :], xh_nat[:, :])
        nc.gpsimd.tensor_copy(yh_nat[:, :], icy_nat[:, :])
        nc.vector.tensor_sub(yl_nat[:, :], icy_nat[:, :], yh_nat[:, :])

        # vo_nat: col 128c = v (fp16), col 128c+32 = 1.0, rest 0 (M padded to 128)
        nc.gpsimd.memset(vo_nat[:, :].bitcast(mybir.dt.uint32), 0)
        vo3 = vo_nat.rearrange("p (c w) -> p c w", w=128)
        nc.vector.tensor_copy(vo3[:, :, 0], v_nat[:, :])
        nc.vector.memset(vo3[:, :, 32], 1.0)

        # --- nat -> row layout via PE transpose + copy + DMA ---
        with tcx.tile_pool(name="tps", bufs=2, space="PSUM") as tp_pool, \
             tcx.tile_pool(name="tsb", bufs=2) as tsb_pool:

            tp_count = [0]

            def to_rows(nat, ncols, aug, rows):
                ps = tp_pool.tile([128, 128], F16, tag="ps")
                sb = tsb_pool.tile([128, 128], F16, tag="sb")
                nc.tensor.transpose(ps[:ncols, :], nat[:, :], ident[:, :])
                for _ in range(6):
                    wps = warm_ps.tile([128, 512], F32, tag="warm")
                    nc.tensor.matmul(wps[:, :], wsrc[:, 0:128], wsrc[:, :],
                                     start=True, stop=True)
                if tp_count[0] % 2 == 0:
                    nc.vector.tensor_copy(sb[:ncols, :], ps[:ncols, :])
                else:
                    nc.scalar.copy(sb[:ncols, :], ps[:ncols, :])
                tp_count[0] += 1
                for r in rows:
                    nc.sync.dma_start(
                        out=aug[r:r + 1, :].rearrange("r (c p) -> r c p", p=128),
                        in_=sb[:ncols, :],
                    )

            to_rows(t2h_nat, CT, tc_aug, [0])
            to_rows(t2l_nat, CT, tc_aug, [1])
            to_rows(txh_nat, CT, tc_aug, [2, 4])
            to_rows(txl_nat, CT, tc_aug, [3])
            to_rows(tyh_nat, CT, tc_aug, [5, 7])
            to_rows(tyl_nat, CT, tc_aug, [6])

            # rows 0,1 = 1.0: 0x3C00 fp16 pairs as uint32
            nc.vector.memset(ic_aug[0:2, :].bitcast(mybir.dt.uint32), 0x3C003C00)
            to_rows(xh_nat, IC, ic_aug, [2, 3])
            to_rows(xl_nat, IC, ic_aug, [4])
            to_rows(yh_nat, IC, ic_aug, [5, 6])
            to_rows(yl_nat, IC, ic_aug, [7])
            # bridge the transposes -> first-matmul window so HAM stays warm
            for _ in range(25):
                wps = warm_ps.tile([128, 512], F32, tag="warm")
                nc.tensor.matmul(wps[:, :], wsrc[:, 0:128], wsrc[:, :],
                                 start=True, stop=True)

        warm_cm.__exit__(None, None, None)

        # ---- main loop ----
        # F=1024 is the psum sweet spot on trn2: pl [128,1024] f32 = 2 banks
        # x2 bufs + nd [128,1024] x2 bufs = 8 banks total. Every 2 o-chunks we
        # finalize a 32-partition output group (divide + x4 + x10 broadcast)
        # so the tail overlaps the main loop.
        PG = 2 * F // L  # output partitions finalized per 2 o-chunks
        with (
            tcx.tile_pool(name="psum_l", bufs=2, space="PSUM") as pl_pool,
            tcx.tile_pool(name="psum_nd", bufs=2, space="PSUM") as nd_pool,
            tcx.tile_pool(name="w", bufs=3) as w_pool,
            tcx.tile_pool(name="grp", bufs=2) as grp_pool,
        ):
            for oc in range(OC):
                nd = nd_pool.tile([128, F], F32)
                for icc in range(IC):
                    pl = pl_pool.tile([128, F], F32)
                    lhsT1 = ic_aug[:, icc * 128:(icc + 1) * 128]
                    for sub in range(NSUB):
                        nc.tensor.matmul(
                            pl[:, sub * 512:(sub + 1) * 512],
                            lhsT1,
                            tc_aug[:, oc * F + sub * 512: oc * F + (sub + 1) * 512],
                            start=True,
                            stop=True,
                        )
                    w = w_pool.tile([128, F], F16)
                    nc.scalar.activation(
                        w[:, :],
                        pl[:, :],
                        AF.Exp,
                        bias=bias_nat[:, icc:icc + 1],
                        scale=2.0 * GAMMA,
                    )
                    for sub in range(NSUB):
                        nc.tensor.matmul(
                            nd[:, sub * 512:(sub + 1) * 512],
                            vo_nat[:, 128 * icc:128 * icc + 128],
                            w[:, sub * 512:(sub + 1) * 512],
                            start=(icc == 0),
                            stop=(icc == IC - 1),
                        )
                nc.vector.tensor_copy(nd_rows[0:33, oc * F:(oc + 1) * F],
                                      nd[0:33, :])
                # finalize this o-chunk's output partitions right away so
                # only the last chunk's finalize is exposed past the loop
                PG2 = F // L
                c0, c1 = oc * F, (oc + 1) * F
                gnum = grp_pool.tile([PG2, L], F32, tag="gnum")
                gden = grp_pool.tile([PG2, L], F32, tag="gden")
                grep = grp_pool.tile([PG2, 4 * L], F32, tag="grep")
                nc.sync.dma_start(
                    out=gnum[:, :],
                    in_=nd_rows[0:1, c0:c1].rearrange("r (p k) -> r p k", k=L),
                )
                nc.gpsimd.dma_start(
                    out=gden[:, :],
                    in_=nd_rows[32:33, c0:c1].rearrange("r (p k) -> r p k", k=L),
                )
                nc.vector.tensor_scalar_add(gden[:, :], gden[:, :], EPS * WSCALE)
                nc.vector.reciprocal(gden[:, :], gden[:, :])
                nc.vector.tensor_mul(gnum[:, :], gnum[:, :], gden[:, :])
                grep3 = grep.rearrange("p (k t) -> p k t", t=4)
                for t in range(4):
                    nc.vector.tensor_copy(grep3[:, :, t], gnum[:, :])
                engs = [nc.sync, nc.gpsimd]
                for si in range(s):
                    engs[si % len(engs)].dma_start(
                        out=out_h[:][si].rearrange("o t -> (o t)").rearrange(
                            "(p j) -> p j", p=n_out * 4 // (4 * L))[
                                oc * PG2:(oc + 1) * PG2, :],
                        in_=grep[:, :],
                    )


@lru_cache(maxsize=2)
def build_nc(n_in=N_IN, n_out=N_OUT, s=S, F=1024):
    nc = bacc.Bacc("TRN2", target_bir_lowering=False, debug=False)
    dat = nc.dram_tensor("dat", [T_IN, n_in, V_IN], F32, kind="ExternalInput")
    ic_h = nc.dram_tensor("ic", [n_in, 2], F32, kind="ExternalInput")
    tc_h = nc.dram_tensor("tc", [n_out, 2], F32, kind="ExternalInput")
    out_h = nc.dram_tensor("out", [s, n_out, T_OUT], F32, kind="ExternalOutput")
    with tile.TileContext(nc) as tcx:
        build_kernel(tcx, dat, ic_h, tc_h, out_h, n_in, n_out, s, F=F)
    nc.compile()
    return nc


def _run(input_data, input_coords, target_coords, n_samples, trace=False):
    n_samples = int(n_samples)
    assert n_samples == S, f"kernel compiled for n_samples={S}, got {n_samples}"
    assert input_data.shape == (B, T_IN, N_IN, V_IN)
    nc = build_nc()
    in_maps = [
        {
            "dat": np.ascontiguousarray(input_data[b], dtype=np.float32),
            "ic": np.ascontiguousarray(input_coords[b], dtype=np.float32),
            "tc": np.ascontiguousarray(target_coords[b], dtype=np.float32),
        }
        for b in range(B)
    ]
    res = run_bass_kernel_spmd(nc, in_maps, list(range(B)), trace=trace)
    out = np.stack([res.results[b]["out"] for b in range(B)], axis=0)
    return out, res


def kernel(input_data, input_coords, target_coords, n_samples):
    out, _ = _run(
        np.asarray(input_data),
        np.asarray(input_coords),
        np.asarray(target_coords),
        n_samples,
    )
    return out


# revision 29
# speedup vs baseline: 1.0325x; 1.0325x over previous
"""RBF/KNN interpolation kernel for Trainium2 (8 NeuronCores, data parallel).

Computes, per batch b:
    v        = input_data[b, -1, :, 0]                      (N_in,)
    w[o, i]  = exp(-||tc[o] - ic[i]||^2 / (2 * 0.1^2))      (N_out, N_in)
    interp   = (w @ v) / (w.sum(-1) + 1e-8)                 (N_out,)
    out[b]   = broadcast(interp) -> (n_samples, N_out, 4)

Sharding: batch B=8 across 8 cores (one batch per core). The weight matrix
is built on-chip tile by tile (never materialized in HBM):
  - logits psum[i, o] via a K=8 fp16 matmul. fp32 coords are split into
    fp16 (hi, lo) pairs so the single-pass fp16 PE path keeps ~1e-4
    precision on the exponent (fp32 matmul runs 2 passes at half rate):
      cross = xh*txh + xh*txl + xl*txh + (same for y) + 1*t2h + 1*t2l
    where t2h + t2l ~= -0.5 * |tc|^2.
  - w = Exp(100 * logits + bias[i]) on the scalar engine, written as fp16;
    bias = -50*|ic|^2 + 10*ln(2) (the 2^10 factor keeps small weights out
    of the fp16 denormal range; it cancels in num/den).
  - [num; ...; den] += [v, 0 x31, 1].T @ w  (fp16 matmul, fp32 psum accum;
    den lands on psum partition 32 - compute-engine PSUM APs need 32-aligned
    starts).
  - interp = num / (den + 1024e-8), computed in a [128, L] layout, then
    broadcast x4 (vector copies) and x n_samples (DMA) to the output.
"""

from contextlib import ExitStack
from functools import lru_cache

import numpy as np

import concourse.bass as bass
import concourse.bacc as bacc
import concourse.tile as tile
from concourse import mybir
from concourse.bass_utils import run_bass_kernel_spmd

F32 = mybir.dt.float32
F16 = mybir.dt.float16
AF = mybir.ActivationFunctionType
ALU = mybir.AluOpType

# Problem sizes (hardcoded per spec)
B = 8
T_IN = 4
N_IN = 4096
V_IN = 3
N_OUT = 8192
S = 10
T_OUT = 4
GAMMA = 50.0  # 1 / (2 * LENGTH_SCALE^2), LENGTH_SCALE = 0.1
EPS = 1e-8
WSCALE_LOG = 6.93147180559945  # ln(2^10)
WSCALE = 1024.0


def build_kernel(tc_ctx, dat, ic_h, tc_h, out_h, n_in, n_out, s, F=1024):
    tcx = tc_ctx
    nc = tcx.nc
    IC = n_in // 128   # i-chunks
    OC = n_out // F    # o-chunks
    NSUB = F // 512
    L = n_out // 128   # per-partition interp count in output layout
    CT = n_out // 128  # nat-layout columns (target side)

    with ExitStack() as ctx:
        const_pool = ctx.enter_context(tcx.tile_pool(name="const", bufs=1))

        # ---- persistent tiles ----
        # K is zero-padded 8 -> 128: a full-array matmul costs the same cycles
        # (stream rate is per column) but keeps the PE HAM activity monitor
        # seeing a busy array, so the clock un-throttles to 2.4 GHz.
        tc_aug = const_pool.tile([128, n_out], F16)  # rows t2h t2l txh txl txh tyh tyl tyh, rest 0
        ic_aug = const_pool.tile([128, n_in], F16)   # rows 1   1   xh  xh  xl  yh  yh  yl, rest 0
        bias_nat = const_pool.tile([128, IC], F32)
        vo_nat = const_pool.tile([128, 128 * IC], F16)  # [v, 0..., 1@32, 0...] per chunk
        nd_rows = const_pool.tile([33, n_out], F32)  # row 0 = num, row 32 = den
        ident = const_pool.tile([128, 128], F16)

        # ---- head: inputs, identity, splits (all in 128-partition nat layout) ----
        head = ctx.enter_context(tcx.tile_pool(name="head", bufs=1))
        tcx_nat = head.tile([128, CT], F32)
        tcy_nat = head.tile([128, CT], F32)
        icx_nat = head.tile([128, IC], F32)
        icy_nat = head.tile([128, IC], F32)
        v_nat = head.tile([128, IC], F32)

        # PE clock warm-up: the HAM monitor un-throttles (1.2 -> 2.4 GHz)
        # only after ~3.4us of sustained full-array work and re-throttles
        # after ~3.4us idle. Fill the PE's head idle time with dummy
        # full-array matmuls whose source is ready immediately, sized to end
        # roughly when the transpose inputs become ready.
        warm_cm = tcx.tile_pool(name="warm_ps", bufs=2, space="PSUM")
        warm_ps = warm_cm.__enter__()
        wsrc = head.tile([128, 512], F16)
        nc.gpsimd.memset(wsrc[:, :].bitcast(mybir.dt.uint32), 0)
        for _ in range(55):
            wps = warm_ps.tile([128, 512], F32, tag="warm")
            nc.tensor.matmul(wps[:, :], wsrc[:, 0:128], wsrc[:, :],
                             start=True, stop=True)

        # zero the padded-K operands first (they gate the row DMAs);
        # bitcast fp16 pairs to uint32 to halve the element count
        tc_aug_u = tc_aug[:, :].bitcast(mybir.dt.uint32)
        ic_aug_u = ic_aug[:, :].bitcast(mybir.dt.uint32)
        nc.vector.memset(tc_aug_u[:, :tc_aug_u.shape[1] // 2], 0)
        nc.gpsimd.memset(tc_aug_u[:, tc_aug_u.shape[1] // 2:], 0)
        nc.vector.memset(ic_aug_u, 0)

        # coordinate loads, one contiguous nat tile per component
        tc_r = tc_h[:].rearrange("(c p) d -> p c d", p=128)
        ic_r = ic_h[:].rearrange("(c p) d -> p c d", p=128)
        h = CT // 2
        nc.sync.dma_start(out=tcx_nat[:, 0:h].rearrange("p (c o) -> p c o", o=1),
                          in_=tc_r[:, 0:h, 0:1])
        nc.gpsimd.dma_start(out=tcx_nat[:, h:].rearrange("p (c o) -> p c o", o=1),
                            in_=tc_r[:, h:, 0:1])
        nc.scalar.dma_start(out=tcy_nat[:, 0:h].rearrange("p (c o) -> p c o", o=1),
                            in_=tc_r[:, 0:h, 1:2])
        nc.sync.dma_start(out=tcy_nat[:, h:].rearrange("p (c o) -> p c o", o=1),
                          in_=tc_r[:, h:, 1:2])
        nc.gpsimd.dma_start(out=icx_nat.rearrange("p (c o) -> p c o", o=1),
                            in_=ic_r[:, :, 0:1])
        nc.scalar.dma_start(out=icy_nat.rearrange("p (c o) -> p c o", o=1),
                            in_=ic_r[:, :, 1:2])
        nc.sync.dma_start(
            out=v_nat[:, :],
            in_=dat[:][T_IN - 1, :, 0].rearrange("(c p) -> p c", p=128),
        )

        # identity for PE transposes: ident[p, f] = (p == f)
        jj = head.tile([128, 128], F32)
        kk = head.tile([128, 1], F32)
        nc.gpsimd.iota(jj[:, :], [[1, 128]], base=0, channel_multiplier=0,
                       allow_small_or_imprecise_dtypes=True)
        nc.gpsimd.iota(kk[:, :], [[0, 1]], base=0, channel_multiplier=1,
                       allow_small_or_imprecise_dtypes=True)
        nc.gpsimd.tensor_scalar(ident[:, :], jj[:, :], kk[:, 0:1], None,
                                op0=ALU.is_equal)

        # --- target-side nat computes (chunk-major: x[p, c] = f(tc[c*128+p])) ---
        sqx_t = head.tile([128, CT], F32)
        t2s = head.tile([128, CT], F32)
        nc.gpsimd.tensor_mul(sqx_t[:, :], tcx_nat[:, :], tcx_nat[:, :])
        nc.vector.scalar_tensor_tensor(t2s[:, :], tcy_nat[:, :], 1.0,
                                       tcy_nat[:, :], op0=ALU.bypass,
                                       op1=ALU.mult)
        nc.vector.tensor_add(t2s[:, :], t2s[:, :], sqx_t[:, :])

        t2h_nat = head.tile([128, CT], F16)
        t2l_nat = head.tile([128, CT], F16)
        nc.vector.tensor_scalar_mul(t2h_nat[:, :], t2s[:, :], -0.5)
        nc.vector.scalar_tensor_tensor(t2l_nat[:, :], t2s[:, :], -0.5,
                                       t2h_nat[:, :], op0=ALU.mult,
                                       op1=ALU.subtract)
        txh_nat = head.tile([128, CT], F16)
        txl_nat = head.tile([128, CT], F16)
        tyh_nat = head.tile([128, CT], F16)
        tyl_nat = head.tile([128, CT], F16)
        nc.gpsimd.tensor_copy(txh_nat[:, :], tcx_nat[:, :])
        nc.vector.tensor_sub(txl_nat[:, :], tcx_nat[:, :], txh_nat[:, :])
        nc.gpsimd.tensor_copy(tyh_nat[:, :], tcy_nat[:, :])
        nc.vector.tensor_sub(tyl_nat[:, :], tcy_nat[:, :], tyh_nat[:, :])

        # --- input-side nat computes ---
        sqx_i = head.tile([128, IC], F32)
        i2s = head.tile([128, IC], F32)
        nc.gpsimd.tensor_mul(sqx_i[:, :], icx_nat[:, :], icx_nat[:, :])
        nc.vector.scalar_tensor_tensor(i2s[:, :], icy_nat[:, :], 1.0,
                                       icy_nat[:, :], op0=ALU.bypass,
                                       op1=ALU.mult)
        nc.vector.tensor_add(i2s[:, :], i2s[:, :], sqx_i[:, :])
        # bias = -50 * i2 + ln(2^10)
        nc.vector.tensor_scalar(bias_nat[:, :], i2s[:, :], -GAMMA, WSCALE_LOG,
                                op0=ALU.mult, op1=ALU.add)

        xh_nat = head.tile([128, IC], F16)
        xl_nat = head.tile([128, IC], F16)
        yh_nat = head.tile([128, IC], F16)
        yl_nat = head.tile([128, IC], F16)
        nc.gpsimd.tensor_copy(xh_nat[:, :], icx_nat[:, :])
        nc.vector.tensor_sub(xl_nat[:, :], icx_nat[:, :], xh_nat[:, :])
        nc.gpsimd.tensor_copy(yh_nat[:, :], icy_nat[:, :])
        nc.vector.tensor_sub(yl_nat[:, :], icy_nat[:, :], yh_nat[:, :])

        # vo_nat: col 128c = v (fp16), col 128c+32 = 1.0, rest 0 (M padded to 128)
        nc.gpsimd.memset(vo_nat[:, :].bitcast(mybir.dt.uint32), 0)
        vo3 = vo_nat.rearrange("p (c w) -> p c w", w=128)
        nc.vector.tensor_copy(vo3[:, :, 0], v_nat[:, :])
        nc.vector.memset(vo3[:, :, 32], 1.0)

        # --- nat -> row layout via PE transpose + copy + DMA ---
        with tcx.tile_pool(name="tps", bufs=2, space="PSUM") as tp_pool, \
             tcx.tile_pool(name="tsb", bufs=2) as tsb_pool:

            tp_count = [0]

            def to_rows(nat, ncols, aug, rows):
                ps = tp_pool.tile([128, 128], F16, tag="ps")
                sb = tsb_pool.tile([128, 128], F16, tag="sb")
                nc.tensor.transpose(ps[:ncols, :], nat[:, :], ident[:, :])
                if tp_count[0] % 2 == 0:
                    nc.vector.tensor_copy(sb[:ncols, :], ps[:ncols, :])
                else:
                    nc.scalar.copy(sb[:ncols, :], ps[:ncols, :])
                tp_count[0] += 1
                for r in rows:
                    nc.sync.dma_start(
                        out=aug[r:r + 1, :].rearrange("r (c p) -> r c p", p=128),
                        in_=sb[:ncols, :],
                    )

            to_rows(t2h_nat, CT, tc_aug, [0])
            to_rows(t2l_nat, CT, tc_aug, [1])
            to_rows(txh_nat, CT, tc_aug, [2, 4])
            to_rows(txl_nat, CT, tc_aug, [3])
            to_rows(tyh_nat, CT, tc_aug, [5, 7])
            to_rows(tyl_nat, CT, tc_aug, [6])

            # rows 0,1 = 1.0: 0x3C00 fp16 pairs as uint32
            nc.vector.memset(ic_aug[0:2, :].bitcast(mybir.dt.uint32), 0x3C003C00)
            to_rows(xh_nat, IC, ic_aug, [2, 3])
            to_rows(xl_nat, IC, ic_aug, [4])
            to_rows(yh_nat, IC, ic_aug, [5, 6])
            to_rows(yl_nat, IC, ic_aug, [7])
            # bridge the transposes -> first-matmul window so HAM stays warm
            for _ in range(20):
                wps = warm_ps.tile([128, 512], F32, tag="warm")
                nc.tensor.matmul(wps[:, :], wsrc[:, 0:128], wsrc[:, :],
                                 start=True, stop=True)

        warm_cm.__exit__(None, None, None)

        # ---- main loop ----
        # F=1024 is the psum sweet spot on trn2: pl [128,1024] f32 = 2 banks
        # x2 bufs + nd [128,1024] x2 bufs = 8 banks total. Every 2 o-chunks we
        # finalize a 32-partition output group (divide + x4 + x10 broadcast)
        # so the tail overlaps the main loop.
        PG = 2 * F // L  # output partitions finalized per 2 o-chunks
        with (
            tcx.tile_pool(name="psum_l", bufs=2, space="PSUM") as pl_pool,
            tcx.tile_pool(name="psum_nd", bufs=2, space="PSUM") as nd_pool,
            tcx.tile_pool(name="w", bufs=3) as w_pool,
            tcx.tile_pool(name="grp", bufs=2) as grp_pool,
        ):
            for oc in range(OC):
                nd = nd_pool.tile([128, F], F32)
                for icc in range(IC):
                    pl = pl_pool.tile([128, F], F32)
                    lhsT1 = ic_aug[:, icc * 128:(icc + 1) * 128]
                    for sub in range(NSUB):
                        nc.tensor.matmul(
                            pl[:, sub * 512:(sub + 1) * 512],
                            lhsT1,
                            tc_aug[:, oc * F + sub * 512: oc * F + (sub + 1) * 512],
                            start=True,
                            stop=True,
                        )
                    w = w_pool.tile([128, F], F16)
                    nc.scalar.activation(
                        w[:, :],
                        pl[:, :],
                        AF.Exp,
                        bias=bias_nat[:, icc:icc + 1],
                        scale=2.0 * GAMMA,
                    )
                    for sub in range(NSUB):
                        nc.tensor.matmul(
                            nd[:, sub * 512:(sub + 1) * 512],
                            vo_nat[:, 128 * icc:128 * icc + 128],
                            w[:, sub * 512:(sub + 1) * 512],
                            start=(icc == 0),
                            stop=(icc == IC - 1),
                        )
                nc.vector.tensor_copy(nd_rows[0:33, oc * F:(oc + 1) * F],
                                      nd[0:33, :])
                # finalize this o-chunk's output partitions right away so
                # only the last chunk's finalize is exposed past the loop
                PG2 = F // L
                c0, c1 = oc * F, (oc + 1) * F
                gnum = grp_pool.tile([PG2, L], F32, tag="gnum")
                gden = grp_pool.tile([PG2, L], F32, tag="gden")
                grep = grp_pool.tile([PG2, 4 * L], F32, tag="grep")
                nc.sync.dma_start(
                    out=gnum[:, :],
                    in_=nd_rows[0:1, c0:c1].rearrange("r (p k) -> r p k", k=L),
                )
                nc.gpsimd.dma_start(
                    out=gden[:, :],
                    in_=nd_rows[32:33, c0:c1].rearrange("r (p k) -> r p k", k=L),
                )
                nc.vector.tensor_scalar_add(gden[:, :], gden[:, :], EPS * WSCALE)
                nc.vector.reciprocal(gden[:, :], gden[:, :])
                nc.vector.tensor_mul(gnum[:, :], gnum[:, :], gden[:, :])
                grep3 = grep.rearrange("p (k t) -> p k t", t=4)
                for t in range(4):
                    nc.vector.tensor_copy(grep3[:, :, t], gnum[:, :])
                engs = [nc.sync, nc.gpsimd]
                for si in range(s):
                    engs[si % len(engs)].dma_start(
                        out=out_h[:][si].rearrange("o t -> (o t)").rearrange(
                            "(p j) -> p j", p=n_out * 4 // (4 * L))[
                                oc * PG2:(oc + 1) * PG2, :],
                        in_=grep[:, :],
                    )


@lru_cache(maxsize=2)
def build_nc(n_in=N_IN, n_out=N_OUT, s=S, F=1024):
    nc = bacc.Bacc("TRN2", target_bir_lowering=False, debug=False)
    dat = nc.dram_tensor("dat", [T_IN, n_in, V_IN], F32, kind="ExternalInput")
    ic_h = nc.dram_tensor("ic", [n_in, 2], F32, kind="ExternalInput")
    tc_h = nc.dram_tensor("tc", [n_out, 2], F32, kind="ExternalInput")
    out_h = nc.dram_tensor("out", [s, n_out, T_OUT], F32, kind="ExternalOutput")
    with tile.TileContext(nc) as tcx:
        build_kernel(tcx, dat, ic_h, tc_h, out_h, n_in, n_out, s, F=F)
    nc.compile()
    return nc


def _run(input_data, input_coords, target_coords, n_samples, trace=False):
    n_samples = int(n_samples)
    assert n_samples == S, f"kernel compiled for n_samples={S}, got {n_samples}"
    assert input_data.shape == (B, T_IN, N_IN, V_IN)
    nc = build_nc()
    in_maps = [
        {
            "dat": np.ascontiguousarray(input_data[b], dtype=np.float32),
            "ic": np.ascontiguousarray(input_coords[b], dtype=np.float32),
            "tc": np.ascontiguousarray(target_coords[b], dtype=np.float32),
        }
        for b in range(B)
    ]
    res = run_bass_kernel_spmd(nc, in_maps, list(range(B)), trace=trace)
    out = np.stack([res.results[b]["out"] for b in range(B)], axis=0)
    return out, res


def kernel(input_data, input_coords, target_coords, n_samples):
    out, _ = _run(
        np.asarray(input_data),
        np.asarray(input_coords),
        np.asarray(target_coords),
        n_samples,
    )
    return out
